# revision 4
# baseline (speedup 1.0000x reference)
"""DeepSeekMoE Trainium2 kernel: 8-way expert-parallel, host-routed dispatch.

v2: all matmul operands in bf16 (fp32 PSUM accumulation), every DMA made
contiguous by host-side pre-transposition into SBUF layout, gate weights
computed on host (exact f32 softmax, free), outputs staged to [128, 2048]
tiles so stores use 4KB partition lines.

Layout notes (per core e):
  - routed expert e computes only its assigned tokens (gathered, zero-padded
    to `cap`), fed as xet [P, DK, cap] bf16 so the contraction dim D lands
    on SBUF partitions with no on-device transposes.
  - the shared SwiGLU is tensor-parallel over the inter dim: each core owns a
    352-wide slice (zero-padded to 384) of sw1/sw3/sw2 and produces a partial
    y over all tokens.
  - wcol [P, capm] f32 carries each gathered token's routing weight.
Host combines: y = sum_e ypart_e; y[idx_e] += yrouted_e[:cnt_e].
"""
import numpy as np

import concourse.bass as bass
import concourse.mybir as mybir
import concourse.tile as tile
from concourse import bacc
from concourse.bass_utils import run_bass_kernel_spmd

D = 2048
F = 1408
E = 8
TOPK = 2
FSL = 352             # per-core shared slice (F * N_SHARED / 8)
FSP = 384             # padded to 3*128
NT = 2048             # tokens (2*1024)
P = 128
DK = D // P           # 16
FK = F // P           # 11
SK = FSP // P         # 3
NQ = 4                # token quarters for the shared expert
QW = NT // NQ         # 512
DCH = 4               # D output chunks of 512
F32 = mybir.dt.float32
BF16 = mybir.dt.bfloat16
NPBF16 = mybir.dt.np(mybir.dt.bfloat16)
SILU = mybir.ActivationFunctionType.Silu

_nc_cache: dict[tuple, object] = {}


def _chunks(total, step):
    out = []
    o = 0
    while o < total:
        out.append((o, min(step, total - o)))
        o += step
    return out


def _build(cap: int, repeat: int = 1):
    """SPMD program for per-expert token capacity `cap` (multiple of 128).

    repeat>1 re-runs the whole body (same inputs -> same outputs); used only
    to amortize the fixed per-dispatch cost when timing.
    """
    from contextlib import ExitStack
    capm = cap // P
    cap_chunks = _chunks(cap, 512)

    nc = bacc.Bacc("TRN2", target_bir_lowering=False)
    xet = nc.declare_dram_parameter("xet", [P, DK, cap], BF16, isOutput=False)
    wcol = nc.declare_dram_parameter("wcol", [P, capm], F32, isOutput=False)
    rw1 = nc.declare_dram_parameter("rw1", [P, FK, DK, P], BF16, isOutput=False)
    rw3 = nc.declare_dram_parameter("rw3", [P, FK, DK, P], BF16, isOutput=False)
    rw2 = nc.declare_dram_parameter("rw2", [P, DCH, FK, 512], BF16,
                                    isOutput=False)
    swa = nc.declare_dram_parameter("swa", [P, DK, FSP], BF16, isOutput=False)
    swb = nc.declare_dram_parameter("swb", [P, DK, FSP], BF16, isOutput=False)
    swc = nc.declare_dram_parameter("swc", [P, SK, D], BF16, isOutput=False)
    xt = nc.declare_dram_parameter("xt", [P, NQ, DK, QW], BF16, isOutput=False)
    yrouted = nc.declare_dram_parameter("yrouted", [capm, P, D], BF16,
                                        isOutput=True)
    ypart = nc.declare_dram_parameter("ypart", [NT // P, P, D], BF16,
                                      isOutput=True)

    with tile.TileContext(nc) as tc, ExitStack() as es:
        res_pool = es.enter_context(tc.tile_pool(name="res", bufs=1))
        WCOL = res_pool.tile([P, capm], F32)
        SWA = res_pool.tile([P, DK, FSP], BF16)
        SWB = res_pool.tile([P, DK, FSP], BF16)
        SWC = res_pool.tile([P, SK, D], BF16)

        for _ in range(repeat):
            # ---- routed phase ----
            with tc.tile_pool(name="gt", bufs=1) as gt_pool, \
                 tc.tile_pool(name="w2res", bufs=1) as w2_pool, \
                 tc.tile_pool(name="stage_rt", bufs=3) as stage, \
                 tc.tile_pool(name="ostage_rt", bufs=2) as ostage, \
                 tc.tile_pool(name="psum_rt", bufs=2, space="PSUM") as psum:
                GT = gt_pool.tile([P, FK, cap], BF16)
                W2 = w2_pool.tile([P, DCH, FK, 512], BF16)

                # layer 1: gT = silu(x@w1) * (x@w3)
                with tc.tile_pool(name="rt1x", bufs=1) as xet_pool, \
                     tc.tile_pool(name="rt1w", bufs=2) as wpool1:
                    nc.sync.dma_start(WCOL[:], wcol[:])
                    XET = xet_pool.tile([P, DK, cap], BF16)
                    for k in range(DK):
                        nc.sync.dma_start(XET[:, k], xet[:, k])
                    for m in range(FK):
                        w1c = wpool1.tile([P, DK, P], BF16, tag="w1c")
                        nc.sync.dma_start(w1c[:], rw1[:, m])
                        w3c = wpool1.tile([P, DK, P], BF16, tag="w3c")
                        nc.sync.dma_start(w3c[:], rw3[:, m])
                        if m >= 4 and m < 4 + DCH:
                            # layer-2 weights + shared residents slot into
                            # the tail of the layer-1 weight stream
                            nc.sync.dma_start(W2[:, m - 4], rw2[:, m - 4])
                        if m == 8:
                            nc.sync.dma_start(SWA[:], swa[:])
                            nc.sync.dma_start(SWB[:], swb[:])
                        if m == 9:
                            nc.sync.dma_start(SWC[:], swc[:])
                        for (n0, nw) in cap_chunks:
                            psa = psum.tile([P, 512], F32, tag="rt1a")
                            for k in range(DK):
                                nc.tensor.matmul(
                                    psa[:, :nw], w1c[:, k],
                                    XET[:, k, n0:n0 + nw],
                                    start=(k == 0), stop=(k == DK - 1))
                            sa = stage.tile([P, 512], F32, tag="rt1_silu")
                            nc.scalar.activation(sa[:, :nw], psa[:, :nw], SILU)
                            psb = psum.tile([P, 512], F32, tag="rt1b")
                            for k in range(DK):
                                nc.tensor.matmul(
                                    psb[:, :nw], w3c[:, k],
                                    XET[:, k, n0:n0 + nw],
                                    start=(k == 0), stop=(k == DK - 1))
                            nc.vector.tensor_mul(GT[:, m, n0:n0 + nw],
                                                 sa[:, :nw], psb[:, :nw])

                # layer 2: y_e[t, :] = w_t * (gT_t @ rw2)
                for mt in range(capm):
                    ot = ostage.tile([P, D], BF16, tag="rt2_out")
                    for ci in range(DCH):
                        ps = psum.tile([P, 512], F32, tag="rt2")
                        for k in range(FK):
                            nc.tensor.matmul(
                                ps[:], GT[:, k, bass.ts(mt, P)],
                                W2[:, ci, k],
                                start=(k == 0), stop=(k == FK - 1))
                        nc.vector.tensor_scalar_mul(
                            ot[:, bass.ts(ci, 512)], ps[:],
                            WCOL[:, mt:mt + 1])
                    nc.scalar.dma_start(yrouted[mt], ot[:])

            # ---- shared expert (TP slice over inter dim) ----
            with tc.tile_pool(name="sh_xt", bufs=2) as xtq_pool, \
                 tc.tile_pool(name="gst", bufs=1) as gst_pool, \
                 tc.tile_pool(name="stage_sh", bufs=3) as stage_sh, \
                 tc.tile_pool(name="ostage_sh", bufs=2) as ostage_sh, \
                 tc.tile_pool(name="psum_sh", bufs=2, space="PSUM") as psum_sh:
                GST = gst_pool.tile([P, SK, NT], BF16)
                for qi in range(NQ):
                    XTQ = xtq_pool.tile([P, DK, QW], BF16, tag="xtq")
                    nc.sync.dma_start(XTQ[:], xt[:, qi])
                    q0 = qi * QW
                    for m in range(SK):
                        psa = psum_sh.tile([P, 512], F32, tag="sh1a")
                        for k in range(DK):
                            nc.tensor.matmul(
                                psa[:], SWA[:, k, bass.ts(m, P)], XTQ[:, k],
                                start=(k == 0), stop=(k == DK - 1))
                        sa = stage_sh.tile([P, 512], F32, tag="sh1_silu")
                        nc.scalar.activation(sa[:], psa[:], SILU)
                        psb = psum_sh.tile([P, 512], F32, tag="sh1b")
                        for k in range(DK):
                            nc.tensor.matmul(
                                psb[:], SWB[:, k, bass.ts(m, P)], XTQ[:, k],
                                start=(k == 0), stop=(k == DK - 1))
                        nc.vector.tensor_mul(GST[:, m, q0:q0 + QW],
                                             sa[:], psb[:])

                    # layer 2 for this quarter's tokens
                    for mt in range(q0 // P, (q0 + QW) // P):
                        ot = ostage_sh.tile([P, D], BF16, tag="sh2_out")
                        for (n0, nw) in _chunks(D, 512):
                            ps = psum_sh.tile([P, 512], F32, tag="sh2")
                            for k in range(SK):
                                nc.tensor.matmul(
                                    ps[:], GST[:, k, bass.ts(mt, P)],
                                    SWC[:, k, n0:n0 + nw],
                                    start=(k == 0), stop=(k == SK - 1))
                            nc.vector.tensor_copy(ot[:, n0:n0 + nw], ps[:])
                        nc.scalar.dma_start(ypart[mt], ot[:])

    nc.compile()
    return nc


def _route(xf: np.ndarray, gate_w: np.ndarray):
    logits = xf @ gate_w
    m = logits.max(-1, keepdims=True)
    ex = np.exp(logits - m)
    scores = ex / ex.sum(-1, keepdims=True)
    top2 = np.argsort(-scores, axis=-1)[:, :TOPK]
    return scores, top2


def _route_idx(xf, gate_w):
    scores, top2 = _route(xf, gate_w)
    idx = [np.where((top2 == e).any(axis=1))[0] for e in range(E)]
    return idx, scores


def build_in_maps(inputs, cap):
    """Per-core device input maps for capacity `cap` (shared with timing)."""
    x = np.asarray(inputs["x"], dtype=np.float32)
    xf = np.ascontiguousarray(x.reshape(-1, D))
    gate_w = np.asarray(inputs["gate_w"], dtype=np.float32)
    idx, scores = _route_idx(xf, gate_w)

    xfb = xf.astype(NPBF16)
    # [P, NQ, DK, QW]: element [p, q, k, u] = xf[q*QW+u, k*128+p]
    xt_b = np.ascontiguousarray(
        xfb.reshape(NQ, QW, DK, P).transpose(3, 0, 2, 1))

    sw1 = np.asarray(inputs["sw1"], dtype=np.float32)
    sw2 = np.asarray(inputs["sw2"], dtype=np.float32)
    sw3 = np.asarray(inputs["sw3"], dtype=np.float32)
    rw1 = np.asarray(inputs["rw1"], dtype=np.float32)
    rw2 = np.asarray(inputs["rw2"], dtype=np.float32)
    rw3 = np.asarray(inputs["rw3"], dtype=np.float32)

    in_maps = []
    for e in range(E):
        ie = idx[e]
        cnt = len(ie)
        # xet [P, DK, cap]: [p, k, c] = xf[ie[c], k*128+p]
        xet = np.zeros((P, DK, cap), dtype=NPBF16)
        xet[:, :, :cnt] = xfb[ie].reshape(cnt, DK, P).transpose(2, 1, 0)
        # routing weight of token c for expert e (host softmax, exact)
        wc = np.zeros(cap, dtype=np.float32)
        wc[:cnt] = scores[ie, e]
        wcol = np.ascontiguousarray(wc.reshape(cap // P, P).T)  # [P, capm]
        # rw1/rw3 [P, FK, DK, P]: [p, m, k, c] = rw[k*128+p, m*128+c]
        rw1_b = np.ascontiguousarray(
            rw1[e].astype(NPBF16).reshape(DK, P, FK, P).transpose(1, 2, 0, 3))
        rw3_b = np.ascontiguousarray(
            rw3[e].astype(NPBF16).reshape(DK, P, FK, P).transpose(1, 2, 0, 3))
        # rw2 [P, DCH, FK, 512]: [p, n, k, c] = rw2[k*128+p, n*512+c]
        rw2_b = np.ascontiguousarray(
            rw2[e].astype(NPBF16).reshape(FK, P, DCH, 512).transpose(1, 2, 0, 3))
        # swa/swb [P, DK, FSP]: [p, k, c] = sw[k*128+p, e*FSL+c] (pad c>=FSL)
        za = np.zeros((D, FSP), dtype=NPBF16)
        za[:, :FSL] = sw1[:, e * FSL:(e + 1) * FSL]
        swa_b = np.ascontiguousarray(
            za.reshape(DK, P, FSP).transpose(1, 0, 2))
        zb = np.zeros((D, FSP), dtype=NPBF16)
        zb[:, :FSL] = sw3[:, e * FSL:(e + 1) * FSL]
        swb_b = np.ascontiguousarray(
            zb.reshape(DK, P, FSP).transpose(1, 0, 2))
        # swc [P, SK, D]: [p, k, c] = sw2[e*FSL + k*128+p, c] (pad)
        zc = np.zeros((FSP, D), dtype=NPBF16)
        zc[:FSL] = sw2[e * FSL:(e + 1) * FSL]
        swc_b = np.ascontiguousarray(
            zc.reshape(SK, P, D).transpose(1, 0, 2))
        in_maps.append({
            "xet": xet, "wcol": wcol,
            "rw1": rw1_b, "rw3": rw3_b, "rw2": rw2_b,
            "swa": swa_b, "swb": swb_b, "swc": swc_b, "xt": xt_b,
        })
    return in_maps, idx


LAST_RESULTS = None


def kernel(x, gate_w, sw1, sw2, sw3, rw1, rw2, rw3, _trace=False):
    x = np.asarray(x, dtype=np.float32)
    B, T, _ = x.shape
    xf = np.ascontiguousarray(x.reshape(-1, D))
    gate_w = np.asarray(gate_w, dtype=np.float32)
    idx, _ = _route_idx(xf, gate_w)
    maxcnt = max(len(i) for i in idx)
    cap = max(512, -(-maxcnt // P) * P)
    if cap not in _nc_cache:
        _nc_cache[cap] = _build(cap)
    nc = _nc_cache[cap]

    in_maps, idx = build_in_maps(
        {"x": x, "gate_w": gate_w, "sw1": sw1, "sw2": sw2, "sw3": sw3,
         "rw1": rw1, "rw2": rw2, "rw3": rw3}, cap)

    res = run_bass_kernel_spmd(nc, in_maps, list(range(E)), trace=_trace)
    global LAST_RESULTS
    LAST_RESULTS = res

    y = res.results[0]["ypart"].astype(np.float32).reshape(NT, D)
    for e in range(1, E):
        y += res.results[e]["ypart"].astype(np.float32).reshape(NT, D)
    for e in range(E):
        ie = idx[e]
        yr = res.results[e]["yrouted"].astype(np.float32).reshape(cap, D)
        y[ie] += yr[:len(ie)]
    return y.reshape(B, T, D)
